# revision 1
# baseline (speedup 1.0000x reference)
"""Multi-head causal attention (B=4, S=4096, E=512, H=8) on 8 trn2 NeuronCores.

Sharding: core = (batch b, head-group g of 4 heads); 4 batches x 2 groups = 8 cores.
Each core computes qkv projection for its group's heads, causal attention, and a
partial output projection (its heads' rows of Wo). Host sums the two partials per
batch and adds bo.

Device layout (per core):
  xT   [512, 4096]   x[b] transposed (host-side) -> contraction dim on partitions
  qT/kT stored [128(2 heads' dh), 1024-token tiles]  (qkvT = W.T @ x.T on PE)
  V    stored token-major [128, kb*260 + h*65 + d] with a ones column per
       (kb, head) at d=64 -> the PV matmul lhsT [Vh|1] yields attention output
       in [dh, tok] layout AND softmax denominators in one pass.
  S_T  [128 keys, 1024 queries] in PSUM per 128-key block, causal-trapezoid
       column ranges; exp on ACT (scale=1/8 folded in); PV accumulates over
       key blocks in PSUM.
"""

import sys

sys.path.insert(0, "/opt/trn_rl_repo")

import numpy as np

B, S, E = 4, 4096, 512
H = 8
DH = 64
HPG = 4  # heads per group
GQ = 256  # features per group for each of q/k/v (HPG*DH)
QE = 1024  # query extent per attention sweep
NQQ = S // QE  # 4
NKB = S // 128  # 32
NTQ = 4  # token chunks for projection phase
TQ = S // NTQ  # 1024
VW = HPG * 65  # 260: per-key-block V width incl. ones columns
NEG = -1.0e10
SCALE = 0.125  # 1/sqrt(DH)

_CACHE = {}


def _chunks(qs, hi):
    """Split [qs, hi) into pieces that never cross a 512-column PSUM bank
    boundary (one matmul output must stay within a single PSUM bank)."""
    out = []
    for c0 in range(0, hi, 512):
        j0, j1 = max(qs, c0), min(hi, c0 + 512)
        if j0 < j1:
            out.append((j0, j1))
    return out



def _build_nc(repeat=1):
    import concourse.bass as bass
    import concourse.tile as tile
    import concourse.mybir as mybir
    from concourse import bacc

    f32 = mybir.dt.float32
    f32r = mybir.dt.float32r
    AF = mybir.ActivationFunctionType
    ALU = mybir.AluOpType

    nc = bacc.Bacc("TRN2", target_bir_lowering=False, debug=False)

    xT = nc.dram_tensor("xT", [E, S], f32r, kind="ExternalInput").ap()
    wqk = nc.dram_tensor("wqk", [E, 512], f32r, kind="ExternalInput").ap()
    bqk = nc.dram_tensor("bqk", [128, 4], f32, kind="ExternalInput").ap()
    wv = nc.dram_tensor("wv", [E, GQ], f32r, kind="ExternalInput").ap()
    bv = nc.dram_tensor("bv", [1, GQ], f32r, kind="ExternalInput").ap()
    wo = nc.dram_tensor("wo", [DH, HPG * 512], f32r, kind="ExternalInput").ap()
    out = nc.dram_tensor("out", [S, E], f32, kind="ExternalOutput").ap()

    with tile.TileContext(nc) as tc:
        with (
            tc.tile_pool(name="consts", bufs=1) as cpool,
            tc.tile_pool(name="xt", bufs=4) as xtpool,
            tc.tile_pool(name="qkv", bufs=1) as qkvpool,
            tc.tile_pool(name="pt", bufs=3) as ptpool,
            tc.tile_pool(name="att", bufs=1) as attpool,
            tc.tile_pool(name="eps", bufs=2) as epool,
            tc.tile_pool(name="outs", bufs=1) as opool,
            # PSUM: 8 banks fully owned by the paired attention loops;
            # projection/Wo psum tiles share the same slots via tags.
            tc.tile_pool(name="st", bufs=1, space="PSUM") as stpool,
            tc.tile_pool(name="ov", bufs=1, space="PSUM") as ovpool,
        ):
            # ---- constants ----
            wqk_sb = cpool.tile([128, 4 * 512], f32r, name="wqk_sb")
            for ec in range(4):
                nc.sync.dma_start(
                    wqk_sb[:, ec * 512 : (ec + 1) * 512],
                    wqk[ec * 128 : (ec + 1) * 128, :],
                )
            wv_sb = cpool.tile([128, 4 * GQ], f32r, name="wv_sb")
            for ec in range(4):
                nc.sync.dma_start(
                    wv_sb[:, ec * GQ : (ec + 1) * GQ],
                    wv[ec * 128 : (ec + 1) * 128, :],
                )
            wo_sb = cpool.tile([DH, HPG * 512], f32r, name="wo_sb")
            nc.sync.dma_start(wo_sb[:], wo[:])
            bqk_sb = cpool.tile([128, 4], f32, name="bqk_sb")
            nc.sync.dma_start(bqk_sb[:], bqk[:])
            bv_sb = cpool.tile([1, GQ], f32r, name="bv_sb")
            nc.sync.dma_start(bv_sb[:], bv[:])
            onesf = cpool.tile([128, 128], f32, name="onesf")
            nc.vector.memset(onesf[:], 1.0)
            ones_row = cpool.tile([1, 128], f32r, name="ones_row")
            nc.vector.tensor_copy(ones_row[:], onesf[0:1, :])
            bf16 = mybir.dt.bfloat16
            maskf = cpool.tile([128, 128], f32, name="maskf")
            nc.vector.memset(maskf[:], 0.0)
            nc.gpsimd.affine_select(
                out=maskf[:], in_=maskf[:], compare_op=ALU.is_ge, fill=NEG,
                base=0, pattern=[[1, 128]], channel_multiplier=-1,
            )
            maskT = cpool.tile([128, 128], bf16, name="maskT")
            nc.vector.tensor_copy(maskT[:], maskf[:])
            identf = cpool.tile([128, 128], f32, name="identf")
            nc.vector.memset(identf[:], 0.0)
            nc.gpsimd.affine_select(
                out=identf[:], in_=identf[:], compare_op=ALU.not_equal, fill=1.0,
                base=0, pattern=[[-1, 128]], channel_multiplier=1,
            )
            ident = cpool.tile([128, 128], bf16, name="ident")
            nc.vector.tensor_copy(ident[:], identf[:])

            # persistent qT/kT tiles: [pair A/B][tq] each [128, 1024]
            # pair A rows 0:64 = head0 dh, 64:128 = head1; pair B = heads 2,3
            qt = [
                [qkvpool.tile([128, TQ], f32r, name=f"qt{ab}_{t}") for t in range(NTQ)]
                for ab in range(2)
            ]
            kt = [
                [qkvpool.tile([128, TQ], f32r, name=f"kt{ab}_{t}") for t in range(NTQ)]
                for ab in range(2)
            ]
            vt = [
                qkvpool.tile([128, 8 * VW], f32r, name=f"vt_{t}") for t in range(NTQ)
            ]

            def p1(tq):
                xts = []
                for ec in range(4):
                    xtile = xtpool.tile([128, TQ], f32r, name="xtile", tag="xtile")
                    nc.sync.dma_start(
                        xtile[:],
                        xT[ec * 128 : (ec + 1) * 128, tq * TQ : (tq + 1) * TQ],
                    )
                    xts.append(xtile)
                for gi, fc in enumerate((0, 2, 1, 3)):
                    dest = (qt if fc < 2 else kt)[fc % 2][tq]
                    for th in range(2):
                        tag = ("st_e", "st_o")[(gi * 2 + th) % 2]
                        ps = stpool.tile([128, 512], f32, name="mmps", tag=tag)
                        for ec in range(4):
                            nc.tensor.matmul(
                                ps[:],
                                lhsT=wqk_sb[:, ec * 512 + fc * 128 : ec * 512 + (fc + 1) * 128],
                                rhs=xts[ec][:, th * 512 : (th + 1) * 512],
                                start=(ec == 0),
                                stop=(ec == 3),
                            )
                        nc.vector.tensor_scalar_add(
                            dest[:, th * 512 : (th + 1) * 512],
                            ps[:],
                            bqk_sb[:, fc : fc + 1],
                        )
                v_tile = vt[tq]
                nc.vector.tensor_copy(
                    v_tile.rearrange("p (t h d) -> p t h d", t=8, h=HPG)[:, :, :, 64:65],
                    onesf[:, 0:32].rearrange("p (t h d) -> p t h d", t=8, h=HPG),
                )
                for tb in range(8):
                    vps = ovpool.tile([128, GQ], f32, name="vps", tag=("ov_e", "ov_o")[tb % 2])
                    for ec in range(4):
                        nc.tensor.matmul(
                            vps[:],
                            lhsT=xts[ec][:, tb * 128 : (tb + 1) * 128],
                            rhs=wv_sb[:, ec * GQ : (ec + 1) * GQ],
                            start=(ec == 0),
                            stop=False,
                        )
                    nc.tensor.matmul(
                        vps[:], lhsT=ones_row[:], rhs=bv_sb[:], start=False, stop=True
                    )
                    nc.vector.tensor_copy(
                        v_tile[:, tb * VW : (tb + 1) * VW].rearrange(
                            "p (h d) -> p h d", h=HPG
                        )[:, :, 0:64],
                        vps.rearrange("p (h d) -> p h d", h=HPG),
                    )

            atts = {}

            def epilogue(oc, ovt):
                # single copy releases the PSUM accumulator; row 64 = sums
                nc.vector.tensor_copy(oc[:], ovt[:])
                sbc = epool.tile([DH, QE], f32, name="sbc", tag="sbc")
                nc.sync.dma_start(
                    sbc[:],
                    oc[64:65, :].bitcast(f32).unsqueeze(1).to_broadcast([1, DH, QE]),
                )
                rbc = epool.tile([DH, QE], f32, name="rbc", tag="rbc")
                scr = epool.tile([DH, QE], f32, name="scr", tag="rscr", bufs=1)
                nc.vector.reciprocal_approx_accurate(out=rbc[:], in_=sbc[:], scratch=scr[:])
                nc.vector.tensor_tensor(oc[0:64, :], oc[0:64, :], rbc[:], ALU.mult)

            def att(qq, mid=None):
                atts[qq] = [
                    attpool.tile([65, QE], f32r, name=f"att_h{h}", tag=f"att{h}")
                    for h in range(HPG)
                ]
                nkb = 8 * qq + 8
                for pr in range(2):  # head pair (2pr, 2pr+1)
                    if pr == 1 and mid is not None:
                        mid()
                    ov_e = ovpool.tile([65, QE], f32, name="ov_e", tag="ov_e")
                    ov_o = ovpool.tile([65, QE], f32, name="ov_o", tag="ov_o")
                    for kb in range(nkb):
                        tqk, kbl = kb // 8, kb % 8
                        qs = max(0, (kb - 8 * qq) * 128)
                        st_e = stpool.tile([128, QE], f32, name="st_e", tag="st_e")
                        st_o = stpool.tile([128, QE], f32, name="st_o", tag="st_o")
                        for j0, j1 in _chunks(qs, QE):
                            # two concurrent row-tiled matmuls (rows 0:64 / 64:128)
                            nc.tensor.matmul(
                                st_e[:, j0:j1],
                                lhsT=kt[pr][tqk][0:64, kbl * 128 : (kbl + 1) * 128],
                                rhs=qt[pr][qq][0:64, j0:j1],
                                start=True,
                                stop=True,
                            )
                            nc.tensor.matmul(
                                st_o[:, j0:j1],
                                lhsT=kt[pr][tqk][64:128, kbl * 128 : (kbl + 1) * 128],
                                rhs=qt[pr][qq][64:128, j0:j1],
                                start=True,
                                stop=True,
                            )
                        if kb >= 8 * qq:  # diagonal: accumulate causal mask on PE
                            for stx in (st_e, st_o):
                                nc.tensor.matmul(
                                    stx[:, qs : qs + 128],
                                    lhsT=ident[:],
                                    rhs=maskT[:],
                                    start=False,
                                    stop=True,
                                    skip_group_check=True,
                                )
                        pt_e = ptpool.tile([128, QE], f32r, name="pt_e", tag="pt")
                        pt_o = ptpool.tile([128, QE], f32r, name="pt_o", tag="pt")
                        nc.scalar.activation(
                            pt_e[:, qs:QE], st_e[:, qs:QE], AF.Exp, bias=0.0, scale=SCALE
                        )
                        nc.scalar.activation(
                            pt_o[:, qs:QE], st_o[:, qs:QE], AF.Exp, bias=0.0, scale=SCALE
                        )
                        for j0, j1 in _chunks(qs, QE):
                            nc.tensor.matmul(
                                ov_e[:, j0:j1],
                                lhsT=vt[tqk][:, kbl * VW + 2 * pr * 65 : kbl * VW + (2 * pr + 1) * 65],
                                rhs=pt_e[:, j0:j1],
                                start=(kb == 0),
                                stop=(kb == nkb - 1),
                                skip_group_check=True,
                            )
                            nc.tensor.matmul(
                                ov_o[:, j0:j1],
                                lhsT=vt[tqk][:, kbl * VW + (2 * pr + 1) * 65 : kbl * VW + (2 * pr + 2) * 65],
                                rhs=pt_o[:, j0:j1],
                                start=(kb == 0),
                                stop=(kb == nkb - 1),
                                skip_group_check=True,
                            )
                    epilogue(atts[qq][2 * pr], ov_e)
                    epilogue(atts[qq][2 * pr + 1], ov_o)

            def wo(qq):
                att_h = atts[qq]
                out_sb = opool.tile([128, 4 * 512], f32, name="out_sb", tag="osb")
                for half in range(2):
                    for tb4 in range(4):
                        tb = half * 4 + tb4
                        wops = stpool.tile(
                            [128, 512], f32, name="wops", tag=("st_e", "st_o")[tb4 % 2]
                        )
                        for h in range(HPG):
                            nc.tensor.matmul(
                                wops[:],
                                lhsT=att_h[h][0:64, tb * 128 : (tb + 1) * 128],
                                rhs=wo_sb[:, h * 512 : (h + 1) * 512],
                                start=(h == 0),
                                stop=(h == HPG - 1),
                            )
                        nc.vector.tensor_copy(
                            out_sb[:, tb4 * 512 : (tb4 + 1) * 512], wops[:]
                        )
                    nc.sync.dma_start(
                        out[
                            qq * QE + half * 512 : qq * QE + (half + 1) * 512, :
                        ].rearrange("(t p) c -> p t c", p=128),
                        out_sb.rearrange("p (t c) -> p t c", t=4),
                    )

            def body(_i=None):
                for tq in range(NTQ):
                    p1(tq)
                att(0)
                att(1, mid=lambda: wo(0))
                att(2, mid=lambda: wo(1))
                att(3, mid=lambda: wo(2))
                wo(3)

            if repeat == 1:
                body()
            else:
                with tc.For_i(0, repeat, 1) as _i:
                    body(_i)

    nc.finalize()
    return nc


def _get_nc(repeat=1):
    key = ("nc", repeat)
    if key not in _CACHE:
        _CACHE[key] = _build_nc(repeat)
    return _CACHE[key]


def _make_in_maps(x, Wqkv, bqkv, Wo):
    in_maps = []
    for core in range(8):
        b, g = core // 2, core % 2
        qs, ks, vs = g * GQ, 512 + g * GQ, 1024 + g * GQ
        wqk_np = np.ascontiguousarray(
            np.concatenate([Wqkv[:, qs : qs + GQ], Wqkv[:, ks : ks + GQ]], axis=1)
        )
        bqk_np = np.ascontiguousarray(
            np.concatenate([bqkv[qs : qs + GQ], bqkv[ks : ks + GQ]]).reshape(4, 128).T
        )
        wv_np = np.ascontiguousarray(Wqkv[:, vs : vs + GQ])
        bv_np = np.ascontiguousarray(bqkv[vs : vs + GQ].reshape(1, GQ))
        wo_g = Wo[g * GQ : (g + 1) * GQ, :]
        wo_np = np.ascontiguousarray(
            np.concatenate([wo_g[h * DH : (h + 1) * DH, :] for h in range(HPG)], axis=1)
        )
        in_maps.append(
            {
                "xT": np.ascontiguousarray(x[b].T),
                "wqk": wqk_np,
                "bqk": bqk_np,
                "wv": wv_np,
                "bv": bv_np,
                "wo": wo_np,
            }
        )
    return in_maps


def kernel(x, Wqkv, bqkv, Wo, bo, **run_kwargs):
    from concourse.bass_utils import run_bass_kernel_spmd

    x = np.asarray(x, dtype=np.float32)
    Wqkv = np.asarray(Wqkv, dtype=np.float32)
    bqkv = np.asarray(bqkv, dtype=np.float32)
    Wo = np.asarray(Wo, dtype=np.float32)
    bo = np.asarray(bo, dtype=np.float32)

    nc = _get_nc()
    in_maps = _make_in_maps(x, Wqkv, bqkv, Wo)

    res = run_bass_kernel_spmd(nc, in_maps, core_ids=list(range(8)), **run_kwargs)
    _CACHE["last_results"] = res

    out = np.empty((B, S, E), dtype=np.float32)
    for b in range(B):
        out[b] = res.results[2 * b]["out"] + res.results[2 * b + 1]["out"] + bo
    return out



# revision 3
# speedup vs baseline: 1.2951x; 1.2951x over previous
"""Multi-head causal attention (B=4, S=4096, E=512, H=8) on 8 trn2 NeuronCores.

Sharding: core = (batch b, head-group g of 4 heads); 4 batches x 2 groups = 8 cores.
Each core computes qkv projection for its group's heads, causal attention, and a
partial output projection (its heads' rows of Wo). Host sums the two partials per
batch and adds bo.

v2: bf16 operands everywhere (FWL weight loads, 2x SBUF/DMA traffic), QE=512
query sweeps with a merged e/o score tile [128, 1024] so each key-block
iteration is ONE exp activation; double-buffered score PSUM so QK(kb+1)
overlaps exp(kb); projection and Wo work interleaved into the attention
stream to fill PE gaps while the scalar engine (exp) saturates.

Device layout (per core):
  xT   [512, 4096] bf16   x[b] transposed -> contraction dim on partitions
  qT/kT stored [128(2 heads' dh), 1024-token tiles]
  V    stored token-major [128, kb*260 + h*65 + d] bf16 with a ones column per
       (kb, head) at d=64 -> PV matmul lhsT [Vh|1] yields attention output
       in [dh, tok] layout AND softmax denominators in one pass.
  st   [128 keys, 1024] PSUM per key-block: cols 0:512 = even head of the
       pair, 512:1024 = odd head (QK pair runs row-tile concurrent on PE);
       causal mask accumulated on PE via ident@maskT; ONE exp (scale=1/8
       folded) -> pt bf16; PV accumulates over key blocks in PSUM [65, 512].
"""

import sys

sys.path.insert(0, "/opt/trn_rl_repo")

import numpy as np

B, S, E = 4, 4096, 512
H = 8
DH = 64
HPG = 4  # heads per group
GQ = 256  # features per group for each of q/k/v (HPG*DH)
QE = 512  # query extent per attention sweep
NQ = S // QE  # 8
NTQ = 4  # token chunks for projection phase
TQ = S // NTQ  # 1024
VW = HPG * 65  # 260: per-key-block V width incl. ones columns
NEG = -1.0e10
SCALE = 0.125  # 1/sqrt(DH)

_CACHE = {}


def _build_nc():
    import concourse.bass as bass
    import concourse.tile as tile
    import concourse.mybir as mybir
    from concourse import bacc

    f32 = mybir.dt.float32
    bf = mybir.dt.bfloat16
    AF = mybir.ActivationFunctionType
    ALU = mybir.AluOpType

    nc = bacc.Bacc("TRN2", target_bir_lowering=False, debug=False)

    xT = nc.dram_tensor("xT", [E, S], bf, kind="ExternalInput").ap()
    wqk = nc.dram_tensor("wqk", [E, 512], bf, kind="ExternalInput").ap()
    bqk = nc.dram_tensor("bqk", [128, 4], f32, kind="ExternalInput").ap()
    wv = nc.dram_tensor("wv", [E, GQ], bf, kind="ExternalInput").ap()
    bv = nc.dram_tensor("bv", [1, GQ], bf, kind="ExternalInput").ap()
    wo = nc.dram_tensor("wo", [DH, HPG * 512], bf, kind="ExternalInput").ap()
    out = nc.dram_tensor("out", [S, E], f32, kind="ExternalOutput").ap()

    with tile.TileContext(nc) as tc:
        with (
            tc.tile_pool(name="consts", bufs=1) as cpool,
            tc.tile_pool(name="xt", bufs=2) as xtpool,
            tc.tile_pool(name="qkv", bufs=1) as qkvpool,
            tc.tile_pool(name="pt", bufs=3) as ptpool,
            tc.tile_pool(name="att", bufs=2) as attpool,
            tc.tile_pool(name="eps", bufs=2) as epool,
            tc.tile_pool(name="outs", bufs=2) as opool,
            # PSUM budget (8 banks of 512 f32):
            #   st  [128,1024] x2 bufs = 4 banks
            #   ov_e/ov_o [65,512] x1  = 2 banks
            #   aux [128,512] x2 bufs  = 2 banks (proj + wo matmul groups)
            tc.tile_pool(name="st", bufs=2, space="PSUM") as stpool,
            tc.tile_pool(name="ov", bufs=1, space="PSUM") as ovpool,
            tc.tile_pool(name="aux", bufs=2, space="PSUM") as auxpool,
        ):
            # ---- constants ----
            wqk_sb = cpool.tile([128, 4 * 512], bf, name="wqk_sb")
            for ec in range(4):
                nc.sync.dma_start(
                    wqk_sb[:, ec * 512 : (ec + 1) * 512],
                    wqk[ec * 128 : (ec + 1) * 128, :],
                )
            wv_sb = cpool.tile([128, 4 * GQ], bf, name="wv_sb")
            for ec in range(4):
                nc.sync.dma_start(
                    wv_sb[:, ec * GQ : (ec + 1) * GQ],
                    wv[ec * 128 : (ec + 1) * 128, :],
                )
            wo_sb = cpool.tile([DH, HPG * 512], bf, name="wo_sb")
            nc.sync.dma_start(wo_sb[:], wo[:])
            bqk_sb = cpool.tile([128, 4], f32, name="bqk_sb")
            nc.sync.dma_start(bqk_sb[:], bqk[:])
            bv_sb = cpool.tile([1, GQ], bf, name="bv_sb")
            nc.sync.dma_start(bv_sb[:], bv[:])
            onesf = cpool.tile([128, 128], bf, name="onesf")
            nc.vector.memset(onesf[:], 1.0)
            ones_row = cpool.tile([1, 128], bf, name="ones_row")
            nc.vector.tensor_copy(ones_row[:], onesf[0:1, :])
            maskf = cpool.tile([128, 128], f32, name="maskf")
            nc.vector.memset(maskf[:], 0.0)
            nc.gpsimd.affine_select(
                out=maskf[:], in_=maskf[:], compare_op=ALU.is_ge, fill=NEG,
                base=0, pattern=[[1, 128]], channel_multiplier=-1,
            )
            maskT = cpool.tile([128, 128], bf, name="maskT")
            nc.vector.tensor_copy(maskT[:], maskf[:])
            identf = cpool.tile([128, 128], f32, name="identf")
            nc.vector.memset(identf[:], 0.0)
            nc.gpsimd.affine_select(
                out=identf[:], in_=identf[:], compare_op=ALU.not_equal, fill=1.0,
                base=0, pattern=[[-1, 128]], channel_multiplier=1,
            )
            ident = cpool.tile([128, 128], bf, name="ident")
            nc.vector.tensor_copy(ident[:], identf[:])

            # persistent qT/kT tiles: [pair A/B][tq] each [128, 1024] bf16
            # pair A rows 0:64 = head0 dh, 64:128 = head1; pair B = heads 2,3
            qt = [
                [qkvpool.tile([128, TQ], bf, name=f"qt{ab}_{t}") for t in range(NTQ)]
                for ab in range(2)
            ]
            kt = [
                [qkvpool.tile([128, TQ], bf, name=f"kt{ab}_{t}") for t in range(NTQ)]
                for ab in range(2)
            ]
            vt = [qkvpool.tile([128, 8 * VW], bf, name=f"vt_{t}") for t in range(NTQ)]

            def p1(tq):
                xts = []
                for ec in range(4):
                    xtile = xtpool.tile([128, TQ], bf, name="xtile", tag=f"xt{ec}")
                    nc.sync.dma_start(
                        xtile[:],
                        xT[ec * 128 : (ec + 1) * 128, tq * TQ : (tq + 1) * TQ],
                    )
                    xts.append(xtile)
                for fc in range(4):
                    dest = (qt if fc < 2 else kt)[fc % 2][tq]
                    for th in range(2):
                        ps = auxpool.tile([128, 512], f32, name="pjps", tag="aux")
                        for ec in range(4):
                            nc.tensor.matmul(
                                ps[:],
                                lhsT=wqk_sb[:, ec * 512 + fc * 128 : ec * 512 + (fc + 1) * 128],
                                rhs=xts[ec][:, th * 512 : (th + 1) * 512],
                                start=(ec == 0),
                                stop=(ec == 3),
                            )
                        nc.vector.tensor_scalar_add(
                            dest[:, th * 512 : (th + 1) * 512],
                            ps[:],
                            bqk_sb[:, fc : fc + 1],
                        )
                v_tile = vt[tq]
                nc.vector.tensor_copy(
                    v_tile.rearrange("p (t h d) -> p t h d", t=8, h=HPG)[:, :, :, 64:65],
                    onesf[:, 0:32].rearrange("p (t h d) -> p t h d", t=8, h=HPG),
                )
                for tb in range(8):
                    vps = auxpool.tile([128, GQ], f32, name="vps", tag="aux")
                    for ec in range(4):
                        nc.tensor.matmul(
                            vps[:],
                            lhsT=xts[ec][:, tb * 128 : (tb + 1) * 128],
                            rhs=wv_sb[:, ec * GQ : (ec + 1) * GQ],
                            start=(ec == 0),
                            stop=False,
                        )
                    nc.tensor.matmul(
                        vps[:], lhsT=ones_row[:], rhs=bv_sb[:], start=False, stop=True
                    )
                    nc.vector.tensor_copy(
                        v_tile[:, tb * VW : (tb + 1) * VW].rearrange(
                            "p (h d) -> p h d", h=HPG
                        )[:, :, 0:64],
                        vps.rearrange("p (h d) -> p h d", h=HPG),
                    )

            atts = {}

            def att(qq, pr):
                nkb = 4 * qq + 4
                tqq, qoff = qq // 2, (qq % 2) * QE
                ov_e = ovpool.tile([65, QE], f32, name="ov_e", tag="ov_e")
                ov_o = ovpool.tile([65, QE], f32, name="ov_o", tag="ov_o")
                for kb in range(nkb):
                    tqk, kbl = kb // 8, kb % 8
                    qs = max(0, kb * 128 - qq * QE)
                    diag = kb >= 4 * qq
                    st = stpool.tile([128, 2 * QE], f32, name="st", tag="st")
                    # QK^T for the head pair: rows 0:64 (even) and 64:128 (odd)
                    # use disjoint PE row groups -> concurrent matmuls.
                    nc.tensor.matmul(
                        st[:, qs:QE],
                        lhsT=kt[pr][tqk][0:64, kbl * 128 : (kbl + 1) * 128],
                        rhs=qt[pr][tqq][0:64, qoff + qs : qoff + QE],
                        start=True,
                        stop=not diag,
                    )
                    nc.tensor.matmul(
                        st[:, QE + qs : 2 * QE],
                        lhsT=kt[pr][tqk][64:128, kbl * 128 : (kbl + 1) * 128],
                        rhs=qt[pr][tqq][64:128, qoff + qs : qoff + QE],
                        start=True,
                        stop=not diag,
                    )
                    if diag:
                        for half in range(2):
                            nc.tensor.matmul(
                                st[:, half * QE + qs : half * QE + qs + 128],
                                lhsT=ident[:],
                                rhs=maskT[:],
                                start=False,
                                stop=True,
                                skip_group_check=True,
                            )
                    pt = ptpool.tile([128, 2 * QE], bf, name="pt", tag="pt")
                    if qs == 0:
                        nc.scalar.activation(
                            pt[:], st[:], AF.Exp, bias=0.0, scale=SCALE
                        )
                    else:
                        st3 = st.rearrange("p (t c) -> p t c", t=2)[:, :, qs:QE]
                        pt3 = pt.rearrange("p (t c) -> p t c", t=2)[:, :, qs:QE]
                        nc.scalar.activation(pt3, st3, AF.Exp, bias=0.0, scale=SCALE)
                    nc.tensor.matmul(
                        ov_e[:, qs:QE],
                        lhsT=vt[tqk][:, kbl * VW + 2 * pr * 65 : kbl * VW + (2 * pr + 1) * 65],
                        rhs=pt[:, qs:QE],
                        start=(kb == 0),
                        stop=(kb == nkb - 1),
                        skip_group_check=True,
                    )
                    nc.tensor.matmul(
                        ov_o[:, qs:QE],
                        lhsT=vt[tqk][:, kbl * VW + (2 * pr + 1) * 65 : kbl * VW + (2 * pr + 2) * 65],
                        rhs=pt[:, QE + qs : 2 * QE],
                        start=(kb == 0),
                        stop=(kb == nkb - 1),
                        skip_group_check=True,
                    )
                # epilogue: normalize by the denominators in row 64
                for half, ov in ((0, ov_e), (1, ov_o)):
                    h = 2 * pr + half
                    ah = attpool.tile([DH, QE], bf, name=f"att{h}", tag=f"att{h}")
                    den_sb = epool.tile([1, QE], f32, name="den", tag=f"den{h}")
                    nc.vector.tensor_copy(den_sb[:], ov[64:65, :])
                    rec = epool.tile([1, QE], f32, name="rec", tag=f"rec{h}")
                    scr = epool.tile([1, QE], f32, name="scr", tag=f"scr{h}")
                    nc.vector.reciprocal_approx_accurate(
                        out=rec[:], in_=den_sb[:], scratch=scr[:]
                    )
                    rb = epool.tile([DH, QE], f32, name="rb", tag=f"rb{h}")
                    nc.sync.dma_start(
                        rb[:], rec.unsqueeze(1).to_broadcast([1, DH, QE])
                    )
                    nc.vector.tensor_tensor(ah[:], ov[0:DH, :], rb[:], ALU.mult)
                    atts[(qq, h)] = ah

            def wo_out(qq):
                out_sb = opool.tile([128, 4 * 512], f32, name="out_sb", tag="osb")
                for tb4 in range(4):
                    wops = auxpool.tile([128, 512], f32, name="wops", tag="aux")
                    for h in range(HPG):
                        nc.tensor.matmul(
                            wops[:],
                            lhsT=atts[(qq, h)][:, tb4 * 128 : (tb4 + 1) * 128],
                            rhs=wo_sb[:, h * 512 : (h + 1) * 512],
                            start=(h == 0),
                            stop=(h == HPG - 1),
                        )
                    nc.vector.tensor_copy(out_sb[:, tb4 * 512 : (tb4 + 1) * 512], wops[:])
                nc.sync.dma_start(
                    out[qq * QE : (qq + 1) * QE, :].rearrange("(t p) c -> p t c", p=128),
                    out_sb.rearrange("p (t c) -> p t c", t=4),
                )

            p1(0)
            for qq in range(NQ):
                att(qq, 0)
                att(qq, 1)
                if qq < NTQ - 1:
                    p1(qq + 1)
                wo_out(qq)

    nc.finalize()
    return nc


def _get_nc():
    if "nc" not in _CACHE:
        _CACHE["nc"] = _build_nc()
    return _CACHE["nc"]


def _make_in_maps(x, Wqkv, bqkv, Wo):
    import ml_dtypes

    bf16 = ml_dtypes.bfloat16
    in_maps = []
    for core in range(8):
        b, g = core // 2, core % 2
        qs, ks, vs = g * GQ, 512 + g * GQ, 1024 + g * GQ
        wqk_np = np.ascontiguousarray(
            np.concatenate([Wqkv[:, qs : qs + GQ], Wqkv[:, ks : ks + GQ]], axis=1)
        ).astype(bf16)
        bqk_np = np.ascontiguousarray(
            np.concatenate([bqkv[qs : qs + GQ], bqkv[ks : ks + GQ]]).reshape(4, 128).T
        )
        wv_np = np.ascontiguousarray(Wqkv[:, vs : vs + GQ]).astype(bf16)
        bv_np = np.ascontiguousarray(bqkv[vs : vs + GQ].reshape(1, GQ)).astype(bf16)
        wo_g = Wo[g * GQ : (g + 1) * GQ, :]
        wo_np = np.ascontiguousarray(
            np.concatenate([wo_g[h * DH : (h + 1) * DH, :] for h in range(HPG)], axis=1)
        ).astype(bf16)
        in_maps.append(
            {
                "xT": np.ascontiguousarray(x[b].T).astype(bf16),
                "wqk": wqk_np,
                "bqk": bqk_np,
                "wv": wv_np,
                "bv": bv_np,
                "wo": wo_np,
            }
        )
    return in_maps


def kernel(x, Wqkv, bqkv, Wo, bo, **run_kwargs):
    from concourse.bass_utils import run_bass_kernel_spmd

    x = np.asarray(x, dtype=np.float32)
    Wqkv = np.asarray(Wqkv, dtype=np.float32)
    bqkv = np.asarray(bqkv, dtype=np.float32)
    Wo = np.asarray(Wo, dtype=np.float32)
    bo = np.asarray(bo, dtype=np.float32)

    nc = _get_nc()
    in_maps = _make_in_maps(x, Wqkv, bqkv, Wo)

    res = run_bass_kernel_spmd(nc, in_maps, core_ids=list(range(8)), **run_kwargs)
    _CACHE["last_results"] = res

    out = np.empty((B, S, E), dtype=np.float32)
    for b in range(B):
        out[b] = res.results[2 * b]["out"] + res.results[2 * b + 1]["out"] + bo
    return out


# revision 5
# speedup vs baseline: 1.3026x; 1.0058x over previous
"""Multi-head causal attention (B=4, S=4096, E=512, H=8) on 8 trn2 NeuronCores.

Sharding: core = (batch b, head-group g of 4 heads); 4 batches x 2 groups = 8 cores.
Each core computes qkv projection for its group's heads, causal attention, and a
partial output projection (its heads' rows of Wo). Host sums the two partials per
batch and adds bo.

v2: bf16 operands everywhere (FWL weight loads, 2x SBUF/DMA traffic), QE=512
query sweeps with a merged e/o score tile [128, 1024] so each key-block
iteration is ONE exp activation; double-buffered score PSUM so QK(kb+1)
overlaps exp(kb); projection and Wo work interleaved into the attention
stream to fill PE gaps while the scalar engine (exp) saturates.

Device layout (per core):
  xT   [512, 4096] bf16   x[b] transposed -> contraction dim on partitions
  qT/kT stored [128(2 heads' dh), 1024-token tiles]
  V    stored token-major [128, kb*260 + h*65 + d] bf16 with a ones column per
       (kb, head) at d=64 -> PV matmul lhsT [Vh|1] yields attention output
       in [dh, tok] layout AND softmax denominators in one pass.
  st   [128 keys, 1024] PSUM per key-block: cols 0:512 = even head of the
       pair, 512:1024 = odd head (QK pair runs row-tile concurrent on PE);
       causal mask accumulated on PE via ident@maskT; ONE exp (scale=1/8
       folded) -> pt bf16; PV accumulates over key blocks in PSUM [65, 512].
"""

import sys

sys.path.insert(0, "/opt/trn_rl_repo")

import numpy as np

B, S, E = 4, 4096, 512
H = 8
DH = 64
HPG = 4  # heads per group
GQ = 256  # features per group for each of q/k/v (HPG*DH)
QE = 512  # query extent per attention sweep
NQ = S // QE  # 8
NTQ = 4  # token chunks for projection phase
TQ = S // NTQ  # 1024
VW = HPG * 65  # 260: per-key-block V width incl. ones columns
NEG = -1.0e10
SCALE = 0.125  # 1/sqrt(DH)

_CACHE = {}


def _build_nc():
    import concourse.bass as bass
    import concourse.tile as tile
    import concourse.mybir as mybir
    from concourse import bacc

    f32 = mybir.dt.float32
    bf = mybir.dt.bfloat16
    AF = mybir.ActivationFunctionType
    ALU = mybir.AluOpType

    nc = bacc.Bacc("TRN2", target_bir_lowering=False, debug=False)

    xT = nc.dram_tensor("xT", [E, S], bf, kind="ExternalInput").ap()
    wqk = nc.dram_tensor("wqk", [E, 512], bf, kind="ExternalInput").ap()
    bqk = nc.dram_tensor("bqk", [128, 4], f32, kind="ExternalInput").ap()
    wv = nc.dram_tensor("wv", [E, GQ], bf, kind="ExternalInput").ap()
    bv = nc.dram_tensor("bv", [1, GQ], bf, kind="ExternalInput").ap()
    wo = nc.dram_tensor("wo", [DH, HPG * 512], bf, kind="ExternalInput").ap()
    out = nc.dram_tensor("out", [S, E], f32, kind="ExternalOutput").ap()

    with tile.TileContext(nc) as tc:
        with (
            tc.tile_pool(name="consts", bufs=1) as cpool,
            tc.tile_pool(name="xt", bufs=2) as xtpool,
            tc.tile_pool(name="qkv", bufs=1) as qkvpool,
            tc.tile_pool(name="pt", bufs=3) as ptpool,
            tc.tile_pool(name="att", bufs=2) as attpool,
            tc.tile_pool(name="eps", bufs=2) as epool,
            tc.tile_pool(name="outs", bufs=2) as opool,
            # PSUM budget (8 banks of 512 f32):
            #   st  [128,1024] x2 bufs = 4 banks
            #   ov_e/ov_o [65,512] x1  = 2 banks
            #   aux [128,512] x2 bufs  = 2 banks (proj + wo matmul groups)
            tc.tile_pool(name="st", bufs=2, space="PSUM") as stpool,
            tc.tile_pool(name="ov", bufs=1, space="PSUM") as ovpool,
            tc.tile_pool(name="aux", bufs=2, space="PSUM") as auxpool,
        ):
            # ---- constants ----
            wqk_sb = cpool.tile([128, 4 * 512], bf, name="wqk_sb")
            for ec in range(4):
                nc.sync.dma_start(
                    wqk_sb[:, ec * 512 : (ec + 1) * 512],
                    wqk[ec * 128 : (ec + 1) * 128, :],
                )
            wv_sb = cpool.tile([128, 4 * GQ], bf, name="wv_sb")
            for ec in range(4):
                nc.sync.dma_start(
                    wv_sb[:, ec * GQ : (ec + 1) * GQ],
                    wv[ec * 128 : (ec + 1) * 128, :],
                )
            wo_sb = cpool.tile([DH, HPG * 512], bf, name="wo_sb")
            nc.sync.dma_start(wo_sb[:], wo[:])
            bqk_sb = cpool.tile([128, 4], f32, name="bqk_sb")
            nc.sync.dma_start(bqk_sb[:], bqk[:])
            bv_sb = cpool.tile([1, GQ], bf, name="bv_sb")
            nc.sync.dma_start(bv_sb[:], bv[:])
            onesf = cpool.tile([128, 128], bf, name="onesf")
            nc.vector.memset(onesf[:], 1.0)
            ones_row = cpool.tile([1, 128], bf, name="ones_row")
            nc.vector.tensor_copy(ones_row[:], onesf[0:1, :])
            maskf = cpool.tile([128, 128], f32, name="maskf")
            nc.vector.memset(maskf[:], 0.0)
            nc.gpsimd.affine_select(
                out=maskf[:], in_=maskf[:], compare_op=ALU.is_ge, fill=NEG,
                base=0, pattern=[[1, 128]], channel_multiplier=-1,
            )
            maskT = cpool.tile([128, 128], bf, name="maskT")
            nc.vector.tensor_copy(maskT[:], maskf[:])
            identf = cpool.tile([128, 128], f32, name="identf")
            nc.vector.memset(identf[:], 0.0)
            nc.gpsimd.affine_select(
                out=identf[:], in_=identf[:], compare_op=ALU.not_equal, fill=1.0,
                base=0, pattern=[[-1, 128]], channel_multiplier=1,
            )
            ident = cpool.tile([128, 128], bf, name="ident")
            nc.vector.tensor_copy(ident[:], identf[:])

            # persistent qT/kT tiles: [pair A/B][tq] each [128, 1024] bf16
            # pair A rows 0:64 = head0 dh, 64:128 = head1; pair B = heads 2,3
            qt = [
                [qkvpool.tile([128, TQ], bf, name=f"qt{ab}_{t}") for t in range(NTQ)]
                for ab in range(2)
            ]
            kt = [
                [qkvpool.tile([128, TQ], bf, name=f"kt{ab}_{t}") for t in range(NTQ)]
                for ab in range(2)
            ]
            vt = [qkvpool.tile([128, 8 * VW], bf, name=f"vt_{t}") for t in range(NTQ)]

            def p1(tq):
                xts = []
                for ec in range(4):
                    xtile = xtpool.tile([128, TQ], bf, name="xtile", tag=f"xt{ec}")
                    nc.sync.dma_start(
                        xtile[:],
                        xT[ec * 128 : (ec + 1) * 128, tq * TQ : (tq + 1) * TQ],
                    )
                    xts.append(xtile)
                for fc in range(4):
                    dest = (qt if fc < 2 else kt)[fc % 2][tq]
                    for th in range(2):
                        ps = auxpool.tile([128, 512], f32, name="pjps", tag="aux")
                        for ec in range(4):
                            nc.tensor.matmul(
                                ps[:],
                                lhsT=wqk_sb[:, ec * 512 + fc * 128 : ec * 512 + (fc + 1) * 128],
                                rhs=xts[ec][:, th * 512 : (th + 1) * 512],
                                start=(ec == 0),
                                stop=(ec == 3),
                            )
                        nc.vector.tensor_scalar_add(
                            dest[:, th * 512 : (th + 1) * 512],
                            ps[:],
                            bqk_sb[:, fc : fc + 1],
                        )
                v_tile = vt[tq]
                nc.vector.tensor_copy(
                    v_tile.rearrange("p (t h d) -> p t h d", t=8, h=HPG)[:, :, :, 64:65],
                    onesf[:, 0:32].rearrange("p (t h d) -> p t h d", t=8, h=HPG),
                )
                for tb in range(8):
                    vps = auxpool.tile([128, GQ], f32, name="vps", tag="aux")
                    for ec in range(4):
                        nc.tensor.matmul(
                            vps[:],
                            lhsT=xts[ec][:, tb * 128 : (tb + 1) * 128],
                            rhs=wv_sb[:, ec * GQ : (ec + 1) * GQ],
                            start=(ec == 0),
                            stop=False,
                        )
                    nc.tensor.matmul(
                        vps[:], lhsT=ones_row[:], rhs=bv_sb[:], start=False, stop=True
                    )
                    nc.vector.tensor_copy(
                        v_tile[:, tb * VW : (tb + 1) * VW].rearrange(
                            "p (h d) -> p h d", h=HPG
                        )[:, :, 0:64],
                        vps.rearrange("p (h d) -> p h d", h=HPG),
                    )

            atts = {}

            def att(qq, pr):
                nkb = 4 * qq + 4
                tqq, qoff = qq // 2, (qq % 2) * QE
                ov_e = ovpool.tile([65, QE], f32, name="ov_e", tag="ov_e")
                ov_o = ovpool.tile([65, QE], f32, name="ov_o", tag="ov_o")

                def pv(kb, pt, qs):
                    tqk, kbl = kb // 8, kb % 8
                    nc.tensor.matmul(
                        ov_e[:, qs:QE],
                        lhsT=vt[tqk][:, kbl * VW + 2 * pr * 65 : kbl * VW + (2 * pr + 1) * 65],
                        rhs=pt[:, qs:QE],
                        start=(kb == 0),
                        stop=(kb == nkb - 1),
                        skip_group_check=True,
                    )
                    nc.tensor.matmul(
                        ov_o[:, qs:QE],
                        lhsT=vt[tqk][:, kbl * VW + (2 * pr + 1) * 65 : kbl * VW + (2 * pr + 2) * 65],
                        rhs=pt[:, QE + qs : 2 * QE],
                        start=(kb == 0),
                        stop=(kb == nkb - 1),
                        skip_group_check=True,
                    )

                prev = None
                for kb in range(nkb):
                    tqk, kbl = kb // 8, kb % 8
                    qs = max(0, kb * 128 - qq * QE)
                    diag = kb >= 4 * qq
                    st = stpool.tile([128, 2 * QE], f32, name="st", tag="st")
                    # QK^T for the head pair: rows 0:64 (even) and 64:128 (odd)
                    # use disjoint PE row groups -> concurrent matmuls.
                    nc.tensor.matmul(
                        st[:, qs:QE],
                        lhsT=kt[pr][tqk][0:64, kbl * 128 : (kbl + 1) * 128],
                        rhs=qt[pr][tqq][0:64, qoff + qs : qoff + QE],
                        start=True,
                        stop=not diag,
                    )
                    nc.tensor.matmul(
                        st[:, QE + qs : 2 * QE],
                        lhsT=kt[pr][tqk][64:128, kbl * 128 : (kbl + 1) * 128],
                        rhs=qt[pr][tqq][64:128, qoff + qs : qoff + QE],
                        start=True,
                        stop=not diag,
                    )
                    if diag:
                        for half in range(2):
                            nc.tensor.matmul(
                                st[:, half * QE + qs : half * QE + qs + 128],
                                lhsT=ident[:],
                                rhs=maskT[:],
                                start=False,
                                stop=True,
                                skip_group_check=True,
                            )
                    pt = ptpool.tile([128, 2 * QE], bf, name="pt", tag="pt")
                    if qs == 0:
                        nc.scalar.activation(
                            pt[:], st[:], AF.Exp, bias=0.0, scale=SCALE
                        )
                    else:
                        st3 = st.rearrange("p (t c) -> p t c", t=2)[:, :, qs:QE]
                        pt3 = pt.rearrange("p (t c) -> p t c", t=2)[:, :, qs:QE]
                        nc.scalar.activation(pt3, st3, AF.Exp, bias=0.0, scale=SCALE)
                    # software pipeline: emit PV for the PREVIOUS kb after this
                    # kb's QK+exp, so the in-order PE queue reaches QK(kb+1)
                    # without stalling on exp(kb).
                    if prev is not None:
                        pv(*prev)
                    prev = (kb, pt, qs)
                pv(*prev)
                # epilogue: copy denominators to SBUF (both heads into one
                # [1, 2*QE] tile), one batched reciprocal, then fused
                # normalize+cast per head.
                den_sb = epool.tile([1, 2 * QE], f32, name="den", tag=f"den{pr}")
                nc.vector.tensor_copy(den_sb[:, 0:QE], ov_e[64:65, :])
                nc.vector.tensor_copy(den_sb[:, QE : 2 * QE], ov_o[64:65, :])
                rec = epool.tile([1, 2 * QE], f32, name="rec", tag=f"rec{pr}")
                scr = epool.tile([1, 2 * QE], f32, name="scr", tag=f"scr{pr}")
                nc.vector.reciprocal_approx_accurate(
                    out=rec[:], in_=den_sb[:], scratch=scr[:]
                )
                for half, ov in ((0, ov_e), (1, ov_o)):
                    h = 2 * pr + half
                    ah = attpool.tile([DH, QE], bf, name=f"att{h}", tag=f"att{h}")
                    rb = epool.tile([DH, QE], f32, name="rb", tag=f"rb{h}")
                    nc.sync.dma_start(
                        rb[:],
                        rec[:, half * QE : (half + 1) * QE]
                        .unsqueeze(1)
                        .to_broadcast([1, DH, QE]),
                    )
                    nc.vector.tensor_tensor(ah[:], ov[0:DH, :], rb[:], ALU.mult)
                    atts[(qq, h)] = ah

            def wo_out(qq):
                out_sb = opool.tile([128, 4 * 512], f32, name="out_sb", tag="osb")
                for tb4 in range(4):
                    wops = auxpool.tile([128, 512], f32, name="wops", tag="aux")
                    for h in range(HPG):
                        nc.tensor.matmul(
                            wops[:],
                            lhsT=atts[(qq, h)][:, tb4 * 128 : (tb4 + 1) * 128],
                            rhs=wo_sb[:, h * 512 : (h + 1) * 512],
                            start=(h == 0),
                            stop=(h == HPG - 1),
                        )
                    nc.vector.tensor_copy(out_sb[:, tb4 * 512 : (tb4 + 1) * 512], wops[:])
                nc.sync.dma_start(
                    out[qq * QE : (qq + 1) * QE, :].rearrange("(t p) c -> p t c", p=128),
                    out_sb.rearrange("p (t c) -> p t c", t=4),
                )

            p1(0)
            for qq in range(NQ):
                att(qq, 0)
                att(qq, 1)
                if qq < NTQ - 1:
                    p1(qq + 1)
                wo_out(qq)

    nc.finalize()
    return nc


def _get_nc():
    if "nc" not in _CACHE:
        _CACHE["nc"] = _build_nc()
    return _CACHE["nc"]


def _make_in_maps(x, Wqkv, bqkv, Wo):
    import ml_dtypes

    bf16 = ml_dtypes.bfloat16
    in_maps = []
    for core in range(8):
        b, g = core // 2, core % 2
        qs, ks, vs = g * GQ, 512 + g * GQ, 1024 + g * GQ
        wqk_np = np.ascontiguousarray(
            np.concatenate([Wqkv[:, qs : qs + GQ], Wqkv[:, ks : ks + GQ]], axis=1)
        ).astype(bf16)
        bqk_np = np.ascontiguousarray(
            np.concatenate([bqkv[qs : qs + GQ], bqkv[ks : ks + GQ]]).reshape(4, 128).T
        )
        wv_np = np.ascontiguousarray(Wqkv[:, vs : vs + GQ]).astype(bf16)
        bv_np = np.ascontiguousarray(bqkv[vs : vs + GQ].reshape(1, GQ)).astype(bf16)
        wo_g = Wo[g * GQ : (g + 1) * GQ, :]
        wo_np = np.ascontiguousarray(
            np.concatenate([wo_g[h * DH : (h + 1) * DH, :] for h in range(HPG)], axis=1)
        ).astype(bf16)
        in_maps.append(
            {
                "xT": np.ascontiguousarray(x[b].T).astype(bf16),
                "wqk": wqk_np,
                "bqk": bqk_np,
                "wv": wv_np,
                "bv": bv_np,
                "wo": wo_np,
            }
        )
    return in_maps


def kernel(x, Wqkv, bqkv, Wo, bo, **run_kwargs):
    from concourse.bass_utils import run_bass_kernel_spmd

    x = np.asarray(x, dtype=np.float32)
    Wqkv = np.asarray(Wqkv, dtype=np.float32)
    bqkv = np.asarray(bqkv, dtype=np.float32)
    Wo = np.asarray(Wo, dtype=np.float32)
    bo = np.asarray(bo, dtype=np.float32)

    nc = _get_nc()
    in_maps = _make_in_maps(x, Wqkv, bqkv, Wo)

    res = run_bass_kernel_spmd(nc, in_maps, core_ids=list(range(8)), **run_kwargs)
    _CACHE["last_results"] = res

    out = np.empty((B, S, E), dtype=np.float32)
    for b in range(B):
        out[b] = res.results[2 * b]["out"] + res.results[2 * b + 1]["out"] + bo
    return out
